# revision 7
# baseline (speedup 1.0000x reference)
"""Trainium2 Bass kernel for HNet attention (B=4, S=2048, H=768, 12 heads, RoPE, causal).

Sharding: 8 cores = 4 batches x 2 head-groups (6 heads each).
Wq/Wk/Wv split column-wise (head axis), Wo row-wise; host sums the two
partial o_proj outputs per batch (the "all-reduce" done at gather time).

Per-core dataflow (all matmuls in float32r = 1 cyc/row on the PE):
  xT [768,2048] (host-transposed) --PE--> Q,K,V natural [2048,384]
  RoPE on Q,K in natural layout (DVE/GPSIMD), PE-transpose -> QT,KT [384,2048]
  scoresT[k,q] = KT_tile.T @ QT  (per head, causal-trimmed strips)
  exp on ScalarE (no max subtraction; scores ~ N(0,1)), diag tiles masked
  PV: lhsT = [V_h | ones] [k,65] -> attn_outT rows 0:64 + softmax sums row 64
  sums -> (SBUF->SBUF DMA gather) -> reciprocal -> K=6 broadcast matmul -> scale
  o_proj: lhsT = attn_outT tiles, rhs = WoT -> out [2048,768] partial
"""

import os
import sys

import numpy as np

sys.path.insert(0, "/opt/trn_rl_repo")

from contextlib import ExitStack

import concourse.bacc as bacc
import concourse.tile as tile
from concourse import mybir
from concourse.bass_utils import run_bass_kernel_spmd

S = 2048
HID = 768
NH = 6            # heads per core
D = 64
F = NH * D        # 384 per-core feature slice
P = 128
SC = S // P       # 16
FC = HID // P     # 6
MC = F // P       # 3
QW = 512          # q strip width
NQ = S // QW      # 4
N_CORES = 8
ROPE_THETA = 10000.0

F32 = mybir.dt.float32
F32R = mybir.dt.float32r
AF = mybir.ActivationFunctionType


def _r(ap):
    """tiles are allocated as float32r already; keep as passthrough."""
    return ap


def _h3(ap):
    """[P, F] -> [P, NH, D] view."""
    return ap.rearrange("p (h d) -> p h d", h=NH)


def build_program():
    nc = bacc.Bacc("TRN2", target_bir_lowering=False, debug=False,
                   num_devices=N_CORES)

    xT_d = nc.dram_tensor("xT", [HID, S], F32R, kind="ExternalInput").ap()
    wqT_d = nc.dram_tensor("wqT", [HID, F], F32R, kind="ExternalInput").ap()
    wkT_d = nc.dram_tensor("wkT", [HID, F], F32R, kind="ExternalInput").ap()
    wvT_d = nc.dram_tensor("wvT", [HID, F], F32R, kind="ExternalInput").ap()
    woT_d = nc.dram_tensor("woT", [F, HID], F32R, kind="ExternalInput").ap()
    cos_d = nc.dram_tensor("cos6", [S, F], F32, kind="ExternalInput").ap()
    sin_d = nc.dram_tensor("sin6", [S, F], F32, kind="ExternalInput").ap()
    tri_d = nc.dram_tensor("tri", [P, P], F32R, kind="ExternalInput").ap()
    eye_d = nc.dram_tensor("eye", [P, P], F32R, kind="ExternalInput").ap()
    e_d = nc.dram_tensor("emat", [NH, F], F32R, kind="ExternalInput").ap()
    on_d = nc.dram_tensor("ones6", [P, NH], F32R, kind="ExternalInput").ap()
    out_d = nc.dram_tensor("out", [S, HID], F32, kind="ExternalOutput").ap()

    with tile.TileContext(nc) as tc, ExitStack() as ctx:
        const_pool = ctx.enter_context(tc.tile_pool(name="const", bufs=1))
        tri_sb = const_pool.tile([P, P], F32R, tag="tri")
        nc.sync.dma_start(tri_sb[:], tri_d[:])
        eye_sb = const_pool.tile([P, P], F32R, tag="eye")
        nc.sync.dma_start(eye_sb[:], eye_d[:])
        e_sb = const_pool.tile([NH, F], F32R, tag="emat")
        nc.sync.dma_start(e_sb[:], e_d[:])
        on_sb = const_pool.tile([P, NH], F32R, tag="ones6")
        nc.sync.dma_start(on_sb[:], on_d[:])

        # persistent per-phase tensors
        qkT_pool = ctx.enter_context(tc.tile_pool(name="qkT", bufs=1))
        qT = [qkT_pool.tile([P, S], F32R, tag=f"qT{m}", name=f"qT{m}") for m in range(MC)]
        kT = [qkT_pool.tile([P, S], F32R, tag=f"kT{m}", name=f"kT{m}") for m in range(MC)]
        v_pool = ctx.enter_context(tc.tile_pool(name="vp", bufs=1))
        v_sb = [v_pool.tile([P, NH * 65], F32R, tag=f"v{s}", name=f"v{s}") for s in range(SC)]
        ao_pool = ctx.enter_context(tc.tile_pool(name="ao", bufs=1))
        aoT = [ao_pool.tile([P, S], F32R, tag=f"ao{m}", name=f"ao{m}") for m in range(MC)]
        woT_pool = ctx.enter_context(tc.tile_pool(name="woT", bufs=1))
        woT = [woT_pool.tile([P, HID], F32R, tag=f"woT{m}", name=f"woT{m}") for m in range(MC)]
        for m in range(MC):
            nc.sync.dma_start(woT[m][:], woT_d[m * P:(m + 1) * P, :])

        # ---------------- phase 1: load + project + rope + transpose --------
        with tc.tile_pool(name="xT", bufs=1) as xT_pool, \
             tc.tile_pool(name="wT", bufs=1) as wT_pool, \
             tc.tile_pool(name="ld", bufs=3) as ld_pool, \
             tc.tile_pool(name="rope", bufs=3) as rope_pool, \
             tc.tile_pool(name="ps_t", bufs=2, space="PSUM") as ps_t, \
             tc.tile_pool(name="ps_p", bufs=2, space="PSUM") as ps_p:

            xT = [xT_pool.tile([P, S], F32R, tag=f"xT{f}", name=f"xT{f}") for f in range(FC)]
            for f in range(FC):
                nc.sync.dma_start(xT[f][:], xT_d[f * P:(f + 1) * P, :])
            wqT = [wT_pool.tile([P, F], F32R, tag=f"wqT{f}", name=f"wqTs{f}") for f in range(FC)]
            wkT = [wT_pool.tile([P, F], F32R, tag=f"wkT{f}", name=f"wkTs{f}") for f in range(FC)]
            wvT = [wT_pool.tile([P, F], F32R, tag=f"wvT{f}", name=f"wvTs{f}") for f in range(FC)]
            for f in range(FC):
                fs = slice(f * P, (f + 1) * P)
                nc.sync.dma_start(wqT[f][:], wqT_d[fs, :])
                nc.sync.dma_start(wkT[f][:], wkT_d[fs, :])
                nc.sync.dma_start(wvT[f][:], wvT_d[fs, :])

            for s in range(SC):
                sl = slice(s * P, (s + 1) * P)
                cs = ld_pool.tile([P, F], F32, tag="cos")
                nc.sync.dma_start(cs[:], cos_d[sl, :])
                sn = ld_pool.tile([P, F], F32, tag="sin")
                nc.sync.dma_start(sn[:], sin_d[sl, :])

                pq = ps_p.tile([P, F], F32, tag="pq")
                pk = ps_p.tile([P, F], F32, tag="pk")
                pv_ = ps_p.tile([P, F], F32, tag="pv")
                for f in range(FC):
                    st, sp = (f == 0), (f == FC - 1)
                    lhs = _r(xT[f][:, sl])
                    nc.tensor.matmul(pq[:], lhs, _r(wqT[f][:]), start=st, stop=sp)
                    nc.tensor.matmul(pk[:], lhs, _r(wkT[f][:]), start=st, stop=sp)
                    nc.tensor.matmul(pv_[:], lhs, _r(wvT[f][:]), start=st, stop=sp)

                # RoPE (natural layout): out = q*cos + rot_half(q)*sin_signed
                for pp, dstT in ((pq, qT), (pk, kT)):
                    t1 = rope_pool.tile([P, F], F32, tag="t1")
                    nc.vector.tensor_mul(t1[:], pp[:], cs[:])
                    t2 = rope_pool.tile([P, F], F32, tag="t2")
                    nc.vector.tensor_mul(_h3(t2)[:, :, 0:32],
                                         _h3(pp)[:, :, 32:64],
                                         _h3(sn)[:, :, 0:32])
                    nc.vector.tensor_mul(_h3(t2)[:, :, 32:64],
                                         _h3(pp)[:, :, 0:32],
                                         _h3(sn)[:, :, 32:64])
                    qr = rope_pool.tile([P, F], F32R, tag="qr")
                    nc.gpsimd.tensor_add(qr[:], t1[:], t2[:])
                    for m in range(MC):
                        pt = ps_t.tile([P, P], F32R, tag="pt")
                        nc.tensor.transpose(_r(pt[:]),
                                            _r(qr[:, m * P:(m + 1) * P]),
                                            _r(eye_sb[:]))
                        nc.vector.tensor_copy(dstT[m][:, sl], pt[:])

                # V with ones column per head: [V_h | 1] -> [P, NH*65]
                v3 = v_sb[s].rearrange("p (h e) -> p h e", h=NH)
                nc.vector.tensor_copy(v3[:, :, 0:64], _h3(pv_[:]))
                nc.vector.tensor_copy(v3[:, :, 64:65],
                                      on_sb.rearrange("p (h o) -> p h o", h=NH))

        # ---------------- phase 2: attention + o_proj -----------------------
        with tc.tile_pool(name="ex", bufs=4) as ex_pool, \
             tc.tile_pool(name="stg", bufs=3) as stg_pool, \
             tc.tile_pool(name="sums", bufs=2) as sums_pool, \
             tc.tile_pool(name="ob", bufs=3) as ob_pool, \
             tc.tile_pool(name="ps_s", bufs=2, space="PSUM") as ps_s, \
             tc.tile_pool(name="ps_pv", bufs=2, space="PSUM") as ps_pv, \
             tc.tile_pool(name="ps_b", bufs=2, space="PSUM") as ps_b, \
             tc.tile_pool(name="ps_f", bufs=1, space="PSUM") as ps_f:

            for qc in range(NQ):
                q0 = qc * QW
                sums = sums_pool.tile([NH, QW], F32, tag="sums")
                inv = sums_pool.tile([NH, QW], F32R, tag="inv")
                for h in range(NH):
                    m, off = h // 2, 64 * (h % 2)
                    pvp = ps_pv.tile([65, QW], F32, tag="pvp")
                    last = 4 * qc + 3
                    for kc in range(last + 1):
                        k0 = kc * P
                        qlo = max(q0, k0)
                        n = q0 + QW - qlo
                        sp = ps_s.tile([P, QW], F32, tag="sc")
                        nc.tensor.matmul(sp[:, 0:n],
                                         _r(kT[m][off:off + 64, k0:k0 + P]),
                                         _r(qT[m][off:off + 64, qlo:qlo + n]),
                                         start=True, stop=True)
                        ex = ex_pool.tile([P, QW], F32R, tag="ex")
                        nc.scalar.activation(ex[:, 0:n], sp[:, 0:n], AF.Exp,
                                             scale=0.125)
                        if k0 >= q0:  # diagonal block: zero k > q
                            nc.gpsimd.tensor_mul(ex[:, 0:P], ex[:, 0:P],
                                                 tri_sb[:])
                        nc.tensor.matmul(pvp[:, qlo - q0:QW],
                                         _r(v_sb[kc][:, h * 65:h * 65 + 65]),
                                         _r(ex[:, 0:n]),
                                         start=(kc == 0), stop=(kc == last))
                    # unnormalized attn rows + sums row
                    nc.vector.tensor_copy(aoT[m][off:off + 64, q0:q0 + QW],
                                          pvp[0:64, :])
                    stg = stg_pool.tile([65, QW], F32, tag="stg")
                    nc.scalar.copy(stg[64:65, :], pvp[64:65, :])
                    nc.sync.dma_start(sums[h:h + 1, :], stg[64:65, :])

                with nc.allow_low_precision(reason="f32r holds fp32-rounded softmax sums"):
                    nc.vector.reciprocal(inv[:, :], sums[:, :])
                for m in range(MC):
                    bp = ps_b.tile([P, QW], F32, tag="bp")
                    nc.tensor.matmul(bp[:], _r(e_sb[:, m * P:(m + 1) * P]),
                                     _r(inv[:, :]), start=True, stop=True)
                    nc.vector.tensor_mul(aoT[m][:, q0:q0 + QW],
                                         aoT[m][:, q0:q0 + QW], bp[:])

                for t in range(QW // P):
                    s0 = q0 + t * P
                    fin = ps_f.tile([P, HID], F32, tag="fin")
                    for m in range(MC):
                        st, sp_ = (m == 0), (m == MC - 1)
                        lhs = _r(aoT[m][:, s0:s0 + P])
                        nc.tensor.matmul(fin[:, 0:QW], lhs, _r(woT[m][:, 0:QW]),
                                         start=st, stop=sp_)
                        nc.tensor.matmul(fin[:, QW:HID], lhs,
                                         _r(woT[m][:, QW:HID]),
                                         start=st, stop=sp_)
                    ob = ob_pool.tile([P, HID], F32, tag="ob")
                    nc.vector.tensor_copy(ob[:], fin[:])
                    nc.sync.dma_start(out_d[s0:s0 + P, :], ob[:])

    nc.compile()
    return nc


def _rope_tables():
    inv_freq = 1.0 / (ROPE_THETA ** (np.arange(0, D, 2, dtype=np.float32) / D))
    t = np.arange(S, dtype=np.float32)
    freqs = np.outer(t, inv_freq)                       # [S, 32]
    emb = np.concatenate([freqs, freqs], axis=-1)       # [S, 64]
    cos = np.cos(emb).astype(np.float32)
    sin = np.sin(emb).astype(np.float32)
    sin_signed = sin.copy()
    sin_signed[:, 0:32] *= -1.0                         # fold rotate_half sign
    cos6 = np.tile(cos, (1, NH)).astype(np.float32)
    sin6 = np.tile(sin_signed, (1, NH)).astype(np.float32)
    return np.ascontiguousarray(cos6), np.ascontiguousarray(sin6)


_STATE = {}


def _get_program():
    if "nc" not in _STATE:
        _STATE["nc"] = build_program()
    return _STATE["nc"]


def _make_in_maps(hidden_states, Wq, Wk, Wv, Wo):
    hs = np.asarray(hidden_states, dtype=np.float32)
    Wq = np.asarray(Wq, dtype=np.float32)
    Wk = np.asarray(Wk, dtype=np.float32)
    Wv = np.asarray(Wv, dtype=np.float32)
    Wo = np.asarray(Wo, dtype=np.float32)

    cos6, sin6 = _rope_tables()
    tri = np.triu(np.ones((P, P), dtype=np.float32))        # j >= i keep
    eye = np.eye(P, dtype=np.float32)
    emat = np.repeat(np.eye(NH, dtype=np.float32), D, axis=1)  # [6, 384]

    in_maps = []
    for c in range(N_CORES):
        b, g = c // 2, c % 2
        cols = slice(g * F, (g + 1) * F)
        in_maps.append({
            "xT": np.ascontiguousarray(hs[b].T),                  # [768, S]
            "wqT": np.ascontiguousarray(Wq[cols, :].T),           # [768, 384]
            "wkT": np.ascontiguousarray(Wk[cols, :].T),
            "wvT": np.ascontiguousarray(Wv[cols, :].T),
            "woT": np.ascontiguousarray(Wo[:, cols].T),           # [384, 768]
            "cos6": cos6,
            "sin6": sin6,
            "tri": tri,
            "eye": eye,
            "emat": emat,
            "ones6": np.ones((P, NH), dtype=np.float32),
        })
    return in_maps


def run(hidden_states, Wq, Wk, Wv, Wo, trace=False, **trace_kw):
    nc = _get_program()
    in_maps = _make_in_maps(hidden_states, Wq, Wk, Wv, Wo)
    res = run_bass_kernel_spmd(nc, in_maps, core_ids=list(range(N_CORES)),
                               trace=trace, **trace_kw)
    B = 4
    out = np.empty((B, S, HID), dtype=np.float32)
    for b in range(B):
        out[b] = res.results[2 * b]["out"] + res.results[2 * b + 1]["out"]
    return out, res


def kernel(hidden_states, Wq, Wk, Wv, Wo):
    out, _ = run(hidden_states, Wq, Wk, Wv, Wo,
                 trace=bool(int(os.environ.get("KERNEL_TRACE", "0"))))
    return out
